# Initial kernel scaffold
#
"""ChannelMoE Trainium2 kernel.

Computes, per batch element b:
    pool   = mean(x[b], axis=-1)                               [C]
    h      = relu(pool[:,None]*w1 + b1)                        [C,4]
    scores = einsum('ij,ioj->io', h, w2) + b2                  [C,C]
    s      = layernorm(scores)*gamma + beta, then / temperature
    mask   = top-4 of each row (ties resolved to lowest index, as
             jax.lax.top_k does) via max8 + match_replace
    W      = softmax of masked s per row (zeros elsewhere)
    out[b] = (W + I) @ x[b]          # identity folds in the +x residual

Sharding: data-parallel over B across 8 NeuronCores (8 batch elements
per core); the small weight-gen / norm params are replicated.

Structure: a single software-pipelined loop over the 8 batch elements.
For each b: two [C, L/2] input DMAs stream x_b into SBUF on the SP
HWDGE ring (inputs own that ring exclusively, so the input stream never
queues behind output stores).  The f32r bitcast on both DMA ends tags
the data as fp32r so the PE runs the channel-mix matmuls at 1
cycle/row (4x the plain-fp32 rate) at full fp32 operand precision.
Pooling is split: half 0 on ACT (in-place copy with accumulate, which
also satisfies the verifier's fp32r-producer rounding rule), half 1 on
DVE (tensor_reduce); DVE runs the score/softmax weight-generation
chain; PE transposes W and runs the matmuls into rotating 2-bank PSUM
tiles; ACT (3/4) and DVE (1/4) drain PSUM into an SBUF staging tile,
and the two [C, L/2] output DMAs go out on the SWDGE ring.  Compute
for element b overlaps the input DMA of b+1 and the output DMA of
b-1; the steady-state marginal cost sits at the PE floor
(64x512 rows/element at 1 cyc/row @ 2.4 GHz).
"""

import numpy as np

import concourse.bacc as bacc
import concourse.bass as bass
import concourse.tile as tile
from concourse import masks, mybir
from concourse.bass_utils import run_bass_kernel_spmd

B, C, L, K = 64, 128, 4096, 4
NCORES = 8
BS = B // NCORES
EPS = 1e-5
F32 = mybir.dt.float32
F32R = mybir.dt.float32r
BF16 = mybir.dt.bfloat16
LH = L // 2      # input/output half: one DMA each
MM = 512         # matmul chunk = one PSUM bank

_NC = None


def _emit(nc, reps=1):
    x = nc.dram_tensor("x", [BS, C, L], F32, kind="ExternalInput").ap()
    w1 = nc.dram_tensor("w1", [C, 4], F32, kind="ExternalInput").ap()
    b1 = nc.dram_tensor("b1", [C, 4], F32, kind="ExternalInput").ap()
    w2 = nc.dram_tensor("w2", [C, C, 4], F32, kind="ExternalInput").ap()
    b2 = nc.dram_tensor("b2", [C, C], F32, kind="ExternalInput").ap()
    gamma = nc.dram_tensor("gamma", [C], F32, kind="ExternalInput").ap()
    beta = nc.dram_tensor("beta", [C], F32, kind="ExternalInput").ap()
    temp = nc.dram_tensor("temperature", [1], F32, kind="ExternalInput").ap()
    out = nc.dram_tensor("out", [BS, C, L], F32, kind="ExternalOutput").ap()

    def bcast_over_partitions(ap, n=C):
        # [F] dram vector -> [n, F] with partition stride 0
        return bass.AP(tensor=ap.tensor, offset=ap.offset, ap=[[0, n]] + list(ap.ap))

    with tile.TileContext(nc) as tc:
        with (
            tc.tile_pool(name="const", bufs=1) as const,
            tc.tile_pool(name="xin", bufs=3) as xin,
            tc.tile_pool(name="oout", bufs=2) as oout,
            tc.tile_pool(name="wg", bufs=2) as wg,
            tc.tile_pool(name="wts", bufs=2) as wts,
            tc.tile_pool(name="sm", bufs=4) as sm,
            tc.tile_pool(name="psmm", bufs=3, space="PSUM") as psmm,
            tc.tile_pool(name="pstr", bufs=2, space="PSUM") as pstr,
        ):
            # ---- one-time constants ----
            w1_sb = const.tile([C, 4], F32)
            nc.sync.dma_start(out=w1_sb, in_=w1)
            b1_sb = const.tile([C, 4], F32)
            nc.sync.dma_start(out=b1_sb, in_=b1)
            w2_sb = const.tile([C, C, 4], F32)
            nc.sync.dma_start(out=w2_sb, in_=w2)
            b2_sb = const.tile([C, C], F32)
            nc.sync.dma_start(out=b2_sb, in_=b2)
            gamma_sb = const.tile([C, C], F32)
            nc.gpsimd.dma_start(out=gamma_sb, in_=bcast_over_partitions(gamma))
            beta_sb = const.tile([C, C], F32)
            nc.gpsimd.dma_start(out=beta_sb, in_=bcast_over_partitions(beta))
            temp_sb = const.tile([C, 1], F32)
            nc.gpsimd.dma_start(out=temp_sb, in_=bcast_over_partitions(temp))

            identity = const.tile([C, C], F32)
            masks.make_identity(nc, identity[:])

            eps_sb = const.tile([C, 1], F32)
            nc.vector.memset(eps_sb, EPS)

            t4p = const.tile([C, 8], F32)
            nc.vector.memset(t4p[:, 4:8], -1e38)

            rtemp = const.tile([C, 1], F32)
            nc.vector.reciprocal(rtemp, temp_sb)
            # fold 1/temperature into gamma/beta, 1/L into w1
            nc.vector.tensor_scalar_mul(gamma_sb[:], gamma_sb[:], rtemp[:, 0:1])
            nc.vector.tensor_scalar_mul(beta_sb[:], beta_sb[:], rtemp[:, 0:1])
            nc.vector.tensor_scalar_mul(w1_sb[:], w1_sb[:], 1.0 / L)


            # `reps` > 1 repeats the whole computation inside one NEFF —
            # used only by the timing harness to isolate exec time from
            # per-dispatch overhead.  The graded kernel uses reps=1.
            for _rep in range(reps):
              for b in range(BS):
                # ---- stream x_b in on the SP ring.  The f32r bitcast on
                # both DMA ends tags the tile as fp32r for the PE at full
                # fp32 precision.  Pool half 0 on ACT (in-place copy with
                # accumulate, f32r out), half 1 on DVE (tensor_reduce) ----
                xhs = []
                pp = sm.tile([C, 2], F32, tag="pp")
                for h2 in range(2):
                    x_h = xin.tile([C, LH], F32, tag=f"x_h{h2}")
                    nc.sync.dma_start(
                        out=x_h[:].bitcast(F32R),
                        in_=x[b, :, h2 * LH : (h2 + 1) * LH].bitcast(F32R))
                    if h2 == 0:
                        nc.scalar.activation(
                            out=x_h[:].bitcast(F32R), in_=x_h[:],
                            func=mybir.ActivationFunctionType.Copy,
                            accum_out=pp[:, 0:1],
                        )
                    else:
                        nc.vector.tensor_reduce(
                            out=pp[:, 1:2], in_=x_h,
                            axis=mybir.AxisListType.X, op=mybir.AluOpType.add,
                        )
                    xhs.append(x_h)
                pool_s = sm.tile([C, 1], F32, tag="pool_s")
                nc.vector.tensor_reduce(
                    out=pool_s, in_=pp, axis=mybir.AxisListType.X,
                    op=mybir.AluOpType.add,
                )

                # ---- h = relu(pool*w1 + b1) ----
                h = sm.tile([C, 4], F32, tag="h")
                nc.vector.tensor_scalar(
                    out=h, in0=w1_sb[:], scalar1=pool_s[:, 0:1], scalar2=None,
                    op0=mybir.AluOpType.mult,
                )
                nc.vector.tensor_add(h, h, b1_sb[:])
                nc.vector.tensor_scalar_max(h, h, 0.0)

                # ---- scores = einsum('ij,ioj->io', h, w2) + b2 ----
                scores = wg.tile([C, C], F32, tag="scores")
                nc.vector.scalar_tensor_tensor(
                    out=scores, in0=w2_sb[:, :, 0], scalar=h[:, 0:1],
                    in1=b2_sb[:], op0=mybir.AluOpType.mult,
                    op1=mybir.AluOpType.add,
                )
                for j in range(1, 4):
                    nc.vector.scalar_tensor_tensor(
                        out=scores, in0=w2_sb[:, :, j], scalar=h[:, j : j + 1],
                        in1=scores, op0=mybir.AluOpType.mult,
                        op1=mybir.AluOpType.add,
                    )

                # ---- layernorm over free dim, * gamma/temp + beta/temp ----
                stats = sm.tile([C, 6], F32, tag="stats")
                nc.vector.bn_stats(out=stats, in_=scores)
                mv = sm.tile([C, 2], F32, tag="mv")
                nc.vector.bn_aggr(out=mv, in_=stats)
                rstd = sm.tile([C, 1], F32, tag="rstd")
                nc.scalar.activation(
                    out=rstd, in_=mv[:, 1:2],
                    func=mybir.ActivationFunctionType.Sqrt,
                    bias=eps_sb[:, 0:1], scale=1.0,
                )
                nc.vector.reciprocal(rstd, rstd)
                snorm = wg.tile([C, C], F32, tag="snorm")
                # ((scores - mu) * gamma') * rstd + beta'  ==
                # (scores - mu)*rstd*gamma' + beta'
                nc.vector.scalar_tensor_tensor(
                    out=snorm, in0=scores, scalar=mv[:, 0:1], in1=gamma_sb[:],
                    op0=mybir.AluOpType.subtract, op1=mybir.AluOpType.mult,
                )
                nc.vector.scalar_tensor_tensor(
                    out=snorm, in0=snorm, scalar=rstd[:, 0:1], in1=beta_sb[:],
                    op0=mybir.AluOpType.mult, op1=mybir.AluOpType.add,
                )

                # ---- top-4 mask with lowest-index tie-break ----
                m8 = sm.tile([C, 8], F32, tag="m8")
                nc.vector.max(out=m8, in_=snorm)
                nc.vector.tensor_copy(t4p[:, 0:4], m8[:, 0:4])
                smarked = wg.tile([C, C], F32, tag="smarked")
                nc.vector.match_replace(
                    out=smarked, in_to_replace=t4p[:], in_values=snorm,
                    imm_value=1e30,
                )

                # ---- masked softmax + identity ----
                negm = sm.tile([C, 1], F32, tag="negm")
                nc.vector.tensor_scalar(
                    out=negm, in0=m8[:, 0:1], scalar1=-1.0, scalar2=None,
                    op0=mybir.AluOpType.mult,
                )
                e = wg.tile([C, C], F32, tag="e")
                nc.scalar.activation(
                    out=e, in_=snorm, func=mybir.ActivationFunctionType.Exp,
                    bias=negm[:, 0:1], scale=1.0,
                )
                den = sm.tile([C, 1], F32, tag="den")
                wun = wg.tile([C, C], F32, tag="wun")
                nc.vector.scalar_tensor_tensor(
                    out=wun, in0=smarked, scalar=1e29, in1=e,
                    op0=mybir.AluOpType.is_ge, op1=mybir.AluOpType.mult,
                    accum_out=den[:, 0:1],
                )
                rden = sm.tile([C, 1], F32, tag="rden")
                nc.vector.reciprocal(rden, den)
                wfin = wg.tile([C, C], F32, tag="wfin")
                nc.vector.scalar_tensor_tensor(
                    out=wfin, in0=wun, scalar=rden[:, 0:1], in1=identity[:],
                    op0=mybir.AluOpType.mult, op1=mybir.AluOpType.add,
                )

                # ---- transpose W' for the matmul stationary operand ----
                wT_ps = pstr.tile([C, C], F32, tag="wT_ps")
                nc.tensor.transpose(wT_ps[:], wfin[:], identity[:])
                wT = wts.tile([C, C], F32R, tag="wT")
                nc.scalar.copy(wT, wT_ps[:])

                # ---- channel-mix matmuls (fp32r, 1 cyc/row) + staged
                # output.  Each PSUM tile spans 2 banks (2 matmuls);
                # one copy drains it (ACT 3 / DVE 1 per element);
                # output halves go out on the SWDGE ring so neither
                # direction queues behind the other ----
                o_b = oout.tile([C, L], F32, tag="o")
                for jc in range(4):
                    pm = psmm.tile([C, 2 * MM], F32, tag="pm")
                    for jj in range(2):
                        k = jc * 2 + jj
                        h2, kk = divmod(k, 4)
                        nc.tensor.matmul(
                            pm[:, jj * MM : (jj + 1) * MM], wT[:],
                            xhs[h2][:, kk * MM : (kk + 1) * MM].bitcast(F32R),
                            start=True, stop=True,
                        )
                    dst = o_b[:, jc * 2 * MM : (jc + 1) * 2 * MM]
                    if jc == 1:
                        nc.vector.tensor_copy(dst, pm[:])
                    else:
                        nc.scalar.copy(dst, pm[:])
                    if jc == 1:
                        nc.gpsimd.dma_start(
                            out=out[b, :, 0:LH], in_=o_b[:, 0:LH])
                    elif jc == 3:
                        nc.gpsimd.dma_start(
                            out=out[b, :, LH:L], in_=o_b[:, LH:L])

    nc.compile()
    return nc


def _get_nc():
    global _NC
    if _NC is None:
        nc = bacc.Bacc("TRN2", target_bir_lowering=False, debug=False)
        _NC = _emit(nc)
    return _NC


def kernel(x, w1, b1, w2, b2, gamma, beta, temperature):
    nc = _get_nc()
    x = np.ascontiguousarray(x, dtype=np.float32)
    rep = {
        "w1": np.ascontiguousarray(w1, dtype=np.float32),
        "b1": np.ascontiguousarray(b1, dtype=np.float32),
        "w2": np.ascontiguousarray(w2, dtype=np.float32),
        "b2": np.ascontiguousarray(b2, dtype=np.float32),
        "gamma": np.ascontiguousarray(gamma, dtype=np.float32),
        "beta": np.ascontiguousarray(beta, dtype=np.float32),
        "temperature": np.ascontiguousarray(temperature, dtype=np.float32),
    }
    in_maps = [
        {"x": x[i * BS : (i + 1) * BS], **rep} for i in range(NCORES)
    ]
    res = run_bass_kernel_spmd(nc, in_maps, core_ids=list(range(NCORES)))
    return np.concatenate([r["out"] for r in res.results], axis=0)



# revision 1
# speedup vs baseline: 1.3909x; 1.3909x over previous
"""ChannelMoE Trainium2 kernel.

Computes, per batch element b:
    pool   = mean(x[b], axis=-1)                               [C]
    h      = relu(pool[:,None]*w1 + b1)                        [C,4]
    scores = einsum('ij,ioj->io', h, w2) + b2                  [C,C]
    s      = layernorm(scores)*gamma + beta, then / temperature
    mask   = top-4 of each row (ties resolved to lowest index, as
             jax.lax.top_k does) via max8 + match_replace
    W      = softmax of masked s per row (zeros elsewhere)
    out[b] = (W + I) @ x[b]          # identity folds in the +x residual

Sharding: data-parallel over B across 8 NeuronCores (8 batch elements
per core); the small weight-gen / norm params are replicated.

Structure: a single software-pipelined loop over the 8 batch elements.
For each b: two [C, L/2] input DMAs stream x_b into SBUF on the SP
HWDGE ring (inputs own that ring exclusively, so the input stream never
queues behind output stores).  The f32r bitcast on both DMA ends tags
the data as fp32r so the PE runs the channel-mix matmuls at 1
cycle/row (4x the plain-fp32 rate) at full fp32 operand precision.
Pooling is split: half 0 on ACT (in-place copy with accumulate, which
also satisfies the verifier's fp32r-producer rounding rule), half 1 on
DVE (tensor_reduce); DVE runs the score/softmax weight-generation
chain; PE transposes W and runs the matmuls into rotating 2-bank PSUM
tiles; ACT (3/4) and DVE (1/4) drain PSUM into an SBUF staging tile,
and the two [C, L/2] output DMAs go out on the SWDGE ring.  Compute
for element b overlaps the input DMA of b+1 and the output DMA of
b-1; the steady-state marginal cost sits at the PE floor
(64x512 rows/element at 1 cyc/row @ 2.4 GHz).
"""

import numpy as np

import concourse.bacc as bacc
import concourse.bass as bass
import concourse.tile as tile
from concourse import masks, mybir
from concourse.bass_utils import run_bass_kernel_spmd

B, C, L, K = 64, 128, 4096, 4
NCORES = 8
BS = B // NCORES
EPS = 1e-5
F32 = mybir.dt.float32
F32R = mybir.dt.float32r
BF16 = mybir.dt.bfloat16
LH = L // 2      # input/output half: one DMA each
MM = 512         # matmul chunk = one PSUM bank

_NC = None


def _emit(nc, reps=1):
    x = nc.dram_tensor("x", [BS, C, L], F32, kind="ExternalInput").ap()
    w1 = nc.dram_tensor("w1", [C, 4], F32, kind="ExternalInput").ap()
    b1 = nc.dram_tensor("b1", [C, 4], F32, kind="ExternalInput").ap()
    w2 = nc.dram_tensor("w2", [C, C, 4], F32, kind="ExternalInput").ap()
    b2 = nc.dram_tensor("b2", [C, C], F32, kind="ExternalInput").ap()
    gamma = nc.dram_tensor("gamma", [C], F32, kind="ExternalInput").ap()
    beta = nc.dram_tensor("beta", [C], F32, kind="ExternalInput").ap()
    temp = nc.dram_tensor("temperature", [1], F32, kind="ExternalInput").ap()
    out = nc.dram_tensor("out", [BS, C, L], F32, kind="ExternalOutput").ap()

    def bcast_over_partitions(ap, n=C):
        # [F] dram vector -> [n, F] with partition stride 0
        return bass.AP(tensor=ap.tensor, offset=ap.offset, ap=[[0, n]] + list(ap.ap))

    with tile.TileContext(nc) as tc:
        with (
            tc.tile_pool(name="const", bufs=1) as const,
            tc.tile_pool(name="xin", bufs=3) as xin,
            tc.tile_pool(name="oout", bufs=2) as oout,
            tc.tile_pool(name="wg", bufs=2) as wg,
            tc.tile_pool(name="wts", bufs=2) as wts,
            tc.tile_pool(name="sm", bufs=4) as sm,
            tc.tile_pool(name="psmm", bufs=3, space="PSUM") as psmm,
            tc.tile_pool(name="pstr", bufs=2, space="PSUM") as pstr,
        ):
            # ---- one-time constants ----
            w1_sb = const.tile([C, 4], F32)
            nc.sync.dma_start(out=w1_sb, in_=w1)
            b1_sb = const.tile([C, 4], F32)
            nc.sync.dma_start(out=b1_sb, in_=b1)
            w2_sb = const.tile([C, C, 4], F32)
            nc.sync.dma_start(out=w2_sb, in_=w2)
            b2_sb = const.tile([C, C], F32)
            nc.sync.dma_start(out=b2_sb, in_=b2)
            gamma_sb = const.tile([C, C], F32)
            nc.gpsimd.dma_start(out=gamma_sb, in_=bcast_over_partitions(gamma))
            beta_sb = const.tile([C, C], F32)
            nc.gpsimd.dma_start(out=beta_sb, in_=bcast_over_partitions(beta))
            temp_sb = const.tile([C, 1], F32)
            nc.gpsimd.dma_start(out=temp_sb, in_=bcast_over_partitions(temp))

            identity = const.tile([C, C], F32)
            masks.make_identity(nc, identity[:])

            eps_sb = const.tile([C, 1], F32)
            nc.vector.memset(eps_sb, EPS)

            t4p = const.tile([C, 8], F32)
            nc.vector.memset(t4p[:, 4:8], -1e38)

            rtemp = const.tile([C, 1], F32)
            nc.vector.reciprocal(rtemp, temp_sb)
            # fold 1/temperature into gamma/beta, 1/L into w1
            nc.vector.tensor_scalar_mul(gamma_sb[:], gamma_sb[:], rtemp[:, 0:1])
            nc.vector.tensor_scalar_mul(beta_sb[:], beta_sb[:], rtemp[:, 0:1])
            nc.vector.tensor_scalar_mul(w1_sb[:], w1_sb[:], 1.0 / L)


            # `reps` > 1 repeats the whole computation inside one NEFF —
            # used only by the timing harness to isolate exec time from
            # per-dispatch overhead.  The graded kernel uses reps=1.
            for _rep in range(reps):
              for b in range(BS):
                # ---- stream x_b in on the SP ring.  The f32r bitcast on
                # both DMA ends tags the tile as fp32r for the PE at full
                # fp32 precision.  Pool half 0 on ACT (in-place copy with
                # accumulate, f32r out), half 1 on DVE (tensor_reduce) ----
                xhs = []
                pp = sm.tile([C, 2], F32, tag="pp")
                for h2 in range(2):
                    x_h = xin.tile([C, LH], F32, tag=f"x_h{h2}")
                    nc.sync.dma_start(
                        out=x_h[:].bitcast(F32R),
                        in_=x[b, :, h2 * LH : (h2 + 1) * LH].bitcast(F32R))
                    if h2 == 0:
                        nc.scalar.activation(
                            out=x_h[:].bitcast(F32R), in_=x_h[:],
                            func=mybir.ActivationFunctionType.Copy,
                            accum_out=pp[:, 0:1],
                        )
                    else:
                        nc.vector.tensor_reduce(
                            out=pp[:, 1:2], in_=x_h,
                            axis=mybir.AxisListType.X, op=mybir.AluOpType.add,
                        )
                    xhs.append(x_h)
                pool_s = sm.tile([C, 1], F32, tag="pool_s")
                nc.vector.tensor_reduce(
                    out=pool_s, in_=pp, axis=mybir.AxisListType.X,
                    op=mybir.AluOpType.add,
                )

                # ---- h = relu(pool*w1 + b1) ----
                h = sm.tile([C, 4], F32, tag="h")
                nc.vector.tensor_scalar(
                    out=h, in0=w1_sb[:], scalar1=pool_s[:, 0:1], scalar2=None,
                    op0=mybir.AluOpType.mult,
                )
                nc.vector.tensor_add(h, h, b1_sb[:])
                nc.vector.tensor_scalar_max(h, h, 0.0)

                # ---- scores = einsum('ij,ioj->io', h, w2) + b2 ----
                scores = wg.tile([C, C], F32, tag="scores")
                nc.vector.scalar_tensor_tensor(
                    out=scores, in0=w2_sb[:, :, 0], scalar=h[:, 0:1],
                    in1=b2_sb[:], op0=mybir.AluOpType.mult,
                    op1=mybir.AluOpType.add,
                )
                for j in range(1, 4):
                    nc.vector.scalar_tensor_tensor(
                        out=scores, in0=w2_sb[:, :, j], scalar=h[:, j : j + 1],
                        in1=scores, op0=mybir.AluOpType.mult,
                        op1=mybir.AluOpType.add,
                    )

                # ---- layernorm over free dim, * gamma/temp + beta/temp ----
                stats = sm.tile([C, 6], F32, tag="stats")
                nc.vector.bn_stats(out=stats, in_=scores)
                mv = sm.tile([C, 2], F32, tag="mv")
                nc.vector.bn_aggr(out=mv, in_=stats)
                rstd = sm.tile([C, 1], F32, tag="rstd")
                nc.scalar.activation(
                    out=rstd, in_=mv[:, 1:2],
                    func=mybir.ActivationFunctionType.Sqrt,
                    bias=eps_sb[:, 0:1], scale=1.0,
                )
                nc.vector.reciprocal(rstd, rstd)
                snorm = wg.tile([C, C], F32, tag="snorm")
                # ((scores - mu) * gamma') * rstd + beta'  ==
                # (scores - mu)*rstd*gamma' + beta'
                nc.vector.scalar_tensor_tensor(
                    out=snorm, in0=scores, scalar=mv[:, 0:1], in1=gamma_sb[:],
                    op0=mybir.AluOpType.subtract, op1=mybir.AluOpType.mult,
                )
                nc.vector.scalar_tensor_tensor(
                    out=snorm, in0=snorm, scalar=rstd[:, 0:1], in1=beta_sb[:],
                    op0=mybir.AluOpType.mult, op1=mybir.AluOpType.add,
                )

                # ---- top-4 mask with lowest-index tie-break ----
                m8 = sm.tile([C, 8], F32, tag="m8")
                nc.vector.max(out=m8, in_=snorm)
                nc.vector.tensor_copy(t4p[:, 0:4], m8[:, 0:4])
                smarked = wg.tile([C, C], F32, tag="smarked")
                nc.vector.match_replace(
                    out=smarked, in_to_replace=t4p[:], in_values=snorm,
                    imm_value=1e30,
                )

                # ---- masked softmax + identity ----
                negm = sm.tile([C, 1], F32, tag="negm")
                nc.vector.tensor_scalar(
                    out=negm, in0=m8[:, 0:1], scalar1=-1.0, scalar2=None,
                    op0=mybir.AluOpType.mult,
                )
                e = wg.tile([C, C], F32, tag="e")
                nc.scalar.activation(
                    out=e, in_=snorm, func=mybir.ActivationFunctionType.Exp,
                    bias=negm[:, 0:1], scale=1.0,
                )
                den = sm.tile([C, 1], F32, tag="den")
                wun = wg.tile([C, C], F32, tag="wun")
                nc.vector.scalar_tensor_tensor(
                    out=wun, in0=smarked, scalar=1e29, in1=e,
                    op0=mybir.AluOpType.is_ge, op1=mybir.AluOpType.mult,
                    accum_out=den[:, 0:1],
                )
                rden = sm.tile([C, 1], F32, tag="rden")
                nc.vector.reciprocal(rden, den)
                wfin = wg.tile([C, C], F32, tag="wfin")
                nc.vector.scalar_tensor_tensor(
                    out=wfin, in0=wun, scalar=rden[:, 0:1], in1=identity[:],
                    op0=mybir.AluOpType.mult, op1=mybir.AluOpType.add,
                )

                # ---- transpose W' for the matmul stationary operand ----
                wT_ps = pstr.tile([C, C], F32, tag="wT_ps")
                nc.tensor.transpose(wT_ps[:], wfin[:], identity[:])
                wT = wts.tile([C, C], F32R, tag="wT")
                nc.scalar.copy(wT, wT_ps[:])

                # ---- channel-mix matmuls (fp32r, 1 cyc/row) + staged
                # output.  Each PSUM tile spans 2 banks (2 matmuls);
                # one copy drains it (ACT 3 / DVE 1 per element);
                # output halves go out on the SWDGE ring so neither
                # direction queues behind the other ----
                o_b = oout.tile([C, L], F32, tag="o")
                for jc in range(4):
                    pm = psmm.tile([C, 2 * MM], F32, tag="pm")
                    for jj in range(2):
                        k = jc * 2 + jj
                        h2, kk = divmod(k, 4)
                        nc.tensor.matmul(
                            pm[:, jj * MM : (jj + 1) * MM], wT[:],
                            xhs[h2][:, kk * MM : (kk + 1) * MM].bitcast(F32R),
                            start=True, stop=True,
                        )
                    dst = o_b[:, jc * 2 * MM : (jc + 1) * 2 * MM]
                    if jc == 1:
                        nc.vector.tensor_copy(dst, pm[:])
                    else:
                        nc.scalar.copy(dst, pm[:])
                    if jc == 1:
                        nc.gpsimd.dma_start(
                            out=out[b, :, 0:LH], in_=o_b[:, 0:LH])
                    elif jc == 3:
                        nc.gpsimd.dma_start(
                            out=out[b, :, LH:L], in_=o_b[:, LH:L])

    nc.compile()
    return nc


def _get_nc():
    global _NC
    if _NC is None:
        nc = bacc.Bacc("TRN2", target_bir_lowering=False, debug=False)
        _NC = _emit(nc)
    return _NC


def kernel(x, w1, b1, w2, b2, gamma, beta, temperature):
    nc = _get_nc()
    x = np.ascontiguousarray(x, dtype=np.float32)
    rep = {
        "w1": np.ascontiguousarray(w1, dtype=np.float32),
        "b1": np.ascontiguousarray(b1, dtype=np.float32),
        "w2": np.ascontiguousarray(w2, dtype=np.float32),
        "b2": np.ascontiguousarray(b2, dtype=np.float32),
        "gamma": np.ascontiguousarray(gamma, dtype=np.float32),
        "beta": np.ascontiguousarray(beta, dtype=np.float32),
        "temperature": np.ascontiguousarray(temperature, dtype=np.float32),
    }
    in_maps = [
        {"x": x[i * BS : (i + 1) * BS], **rep} for i in range(NCORES)
    ]
    res = run_bass_kernel_spmd(nc, in_maps, core_ids=list(range(NCORES)))
    return np.concatenate([r["out"] for r in res.results], axis=0)

